# revision 17
# baseline (speedup 1.0000x reference)
"""DCNv2 deformable PS-RoI pooling on 8 Trainium2 cores.

Sharding: each core holds a 2-image slice of the feature map (images c and
(c+1)%8, channels-last, stacked into a 32768-row pixel table -> int16
indexable) and exactly 128 RoIs assigned by a cyclic load balancer.

Device kernel (per core, RoIs on partitions):
- stage A (DVE): per-sample coords, validity, bilinear weights; per-bin the
  16 samples form a 4x4 tensor grid (x positions shared across sample rows),
  so all samples of one (bin, sample-row) live in one 6-pixel window; the
  x-interpolation folds into 6 coefficient planes C_top/C_bot built by
  iota-compare + reduce.
- per bin: a tiny PE matmul permutes the 8 window row-indices per RoI into
  dma_gather's wrapped-int16 layout; one dma_gather (1024 idx x 6px rows)
  pulls the data; 48 scalar_tensor_tensor ops accumulate coeff * window
  slices into 4 accumulator chains; scale by 1/count; DMA out.
"""

import sys

sys.path.insert(0, "/opt/trn_rl_repo")

import numpy as np

SPATIAL_SCALE = 0.0625
POOLED = 7
SAMPLES = 4
TRANS_STD = 0.1
B, C, H, W = 8, 256, 128, 128
K = 1024
NBIN = POOLED * POOLED          # 49
NS = SAMPLES * SAMPLES          # 16 samples per bin
P = 128                         # partitions == rois per core
NBLK = SAMPLES * 2              # 8 gather rows per bin (4 sample-rows x top/bot)
NIDX = P * NBLK                 # 1024 indices per gather (DGE per-op limit)
WIN = 6                         # window width in pixels
QW = WIN                        # coefficient slots
ROWS_PER_IMG = H * W            # 16384
TBL_ROWS = 2 * ROWS_PER_IMG     # 32768
TBL_PAD = 64
EC = C                          # elems per pixel row
ES = WIN * EC                   # gather elem_size: 6 pixels
MAGIC = 8388608.0               # 2^23
FEAT_FP16 = False


def _assign_rois(batch_idx):
    """Cyclic load balancer: core c serves images c,(c+1)%8; exactly P rois per core."""
    n = np.bincount(batch_idx, minlength=B).astype(np.int64)
    d = n - P
    prefix = np.concatenate([[0], np.cumsum(d)])[:-1]
    s0 = max(0, int(prefix.max()))
    s = s0 - prefix
    assert np.all(s >= 0) and np.all(s <= n), (n, s)
    ids_by_img = [np.where(batch_idx == b)[0] for b in range(B)]
    core_ids = []
    core_base = []
    for c in range(B):
        nxt = (c + 1) % B
        own = ids_by_img[c][s[c]:]
        spill = ids_by_img[nxt][: s[nxt]]
        ids = np.concatenate([own, spill])
        base = np.concatenate(
            [np.zeros(len(own), np.float32), np.full(len(spill), float(ROWS_PER_IMG), np.float32)]
        )
        assert len(ids) == P, (c, len(ids))
        core_ids.append(ids)
        core_base.append(base)
    return core_ids, core_base


def _build_program(mode="full"):
    import concourse.bass as bass
    import concourse.tile as tile
    from concourse import mybir, bacc

    f32 = mybir.dt.float32
    f16 = mybir.dt.float16
    fdt = f16 if FEAT_FP16 else f32
    AT = mybir.AluOpType

    nc = bacc.Bacc("TRN2", target_bir_lowering=False, debug=False)
    feat = nc.dram_tensor("feat", [TBL_ROWS + TBL_PAD, EC], fdt, kind="ExternalInput")
    rois_in = nc.dram_tensor("rois", [P, 5], f32, kind="ExternalInput")
    off_in = nc.dram_tensor("off", [P, 2 * NBIN], f32, kind="ExternalInput")
    base_in = nc.dram_tensor("base", [P, 1], f32, kind="ExternalInput")
    pwg_in = nc.dram_tensor("pwg", [P, NBIN * NS], f32, kind="ExternalInput")
    phg_in = nc.dram_tensor("phg", [P, NBIN * NS], f32, kind="ExternalInput")
    iwg_in = nc.dram_tensor("iwg", [P, NBIN * NS], f32, kind="ExternalInput")
    ihg_in = nc.dram_tensor("ihg", [P, NBIN * NS], f32, kind="ExternalInput")
    mod16_in = nc.dram_tensor("mod16", [P, P], f32, kind="ExternalInput")
    qmask_in = nc.dram_tensor("qmask", [P, P], f32, kind="ExternalInput")
    out_d = nc.dram_tensor("out", [P, NBIN * C], f32, kind="ExternalOutput")

    NSB = NBIN * NS  # 784

    def expand49(ap_2d):
        a = ap_2d
        return bass.AP(a.tensor, a.offset, list(a.ap[:1]) + [[a.ap[1][0], NBIN], [0, NS]])

    with tile.TileContext(nc) as tc:
        with (
            tc.tile_pool(name="keep", bufs=1) as sa,
            tc.tile_pool(name="gather", bufs=6 if FEAT_FP16 else 4) as gp,
            tc.tile_pool(name="idx", bufs=4) as ip,
            tc.tile_pool(name="rp", bufs=3) as rp,
            tc.tile_pool(name="acc", bufs=1) as accp,
            tc.tile_pool(name="stage", bufs=3) as stp,
            tc.tile_pool(name="psum", bufs=4, space="PSUM") as pp,
            tc.tile_pool(name="psumc", bufs=1, space="PSUM") as ppc,
        ):
            v = nc.vector

            mod16 = sa.tile([P, P], f32, tag="mod16")
            nc.gpsimd.dma_start(mod16[:], mod16_in[:, :])
            qm_psum = ppc.tile([P, P], f32, tag="qm")

            ephemeral = tc.tile_pool(name="eph", bufs=1)
            cpool = tp = ephemeral.__enter__()

            qm_sb = cpool.tile([P, P], f32, tag="qmsb")
            nc.gpsimd.dma_start(qm_sb[:], qmask_in[:, :])
            v.tensor_copy(out=qm_psum[:], in_=qm_sb[:])

            pwg = cpool.tile([P, NSB], f32, tag="pwg")
            nc.gpsimd.dma_start(pwg[:], pwg_in[:, :])
            phg = cpool.tile([P, NSB], f32, tag="phg")
            nc.gpsimd.dma_start(phg[:], phg_in[:, :])
            iwg = cpool.tile([P, NSB], f32, tag="iwg")
            nc.gpsimd.dma_start(iwg[:], iwg_in[:, :])
            ihg = cpool.tile([P, NSB], f32, tag="ihg")
            nc.gpsimd.dma_start(ihg[:], ihg_in[:, :])
            rois = cpool.tile([P, 5], f32, tag="rois")
            nc.gpsimd.dma_start(rois[:], rois_in[:, :])
            off = cpool.tile([P, 2 * NBIN], f32, tag="off")
            nc.gpsimd.dma_start(off[:], off_in[:, :])
            basec = cpool.tile([P, 1], f32, tag="basec")
            nc.gpsimd.dma_start(basec[:], base_in[:, :])

            # ---------------- stage A: per-roi scalars ----------------
            S = SPATIAL_SCALE
            sc1 = cpool.tile([P, 16], f32, tag="sc1")

            rsw = sc1[:, 0:1]; rsh = sc1[:, 1:2]; rew = sc1[:, 2:3]; reh = sc1[:, 3:4]
            rw = sc1[:, 4:5]; rh = sc1[:, 5:6]; bw = sc1[:, 6:7]; bh = sc1[:, 7:8]
            sw = sc1[:, 8:9]; sh = sc1[:, 9:10]; rw01 = sc1[:, 10:11]; rh01 = sc1[:, 11:12]

            def rnd(dst, src_col):
                v.tensor_scalar(out=dst, in0=src_col, scalar1=0.5 + MAGIC, scalar2=-MAGIC, op0=AT.add, op1=AT.add)
                v.tensor_scalar(out=dst, in0=dst, scalar1=S, scalar2=-0.5, op0=AT.mult, op1=AT.add)

            rnd(rsw, rois[:, 1:2])
            rnd(rsh, rois[:, 2:3])
            v.tensor_scalar(out=rew, in0=rois[:, 3:4], scalar1=0.5 + MAGIC, scalar2=-MAGIC, op0=AT.add, op1=AT.add)
            v.tensor_scalar(out=rew, in0=rew, scalar1=S, scalar2=S - 0.5, op0=AT.mult, op1=AT.add)
            v.tensor_scalar(out=reh, in0=rois[:, 4:5], scalar1=0.5 + MAGIC, scalar2=-MAGIC, op0=AT.add, op1=AT.add)
            v.tensor_scalar(out=reh, in0=reh, scalar1=S, scalar2=S - 0.5, op0=AT.mult, op1=AT.add)
            v.tensor_tensor(out=rw, in0=rew, in1=rsw, op=AT.subtract)
            v.tensor_scalar(out=rw, in0=rw, scalar1=0.1, scalar2=None, op0=AT.max)
            v.tensor_tensor(out=rh, in0=reh, in1=rsh, op=AT.subtract)
            v.tensor_scalar(out=rh, in0=rh, scalar1=0.1, scalar2=None, op0=AT.max)
            v.tensor_scalar(out=bw, in0=rw, scalar1=1.0 / POOLED, scalar2=None, op0=AT.mult)
            v.tensor_scalar(out=bh, in0=rh, scalar1=1.0 / POOLED, scalar2=None, op0=AT.mult)
            v.tensor_scalar(out=sw, in0=bw, scalar1=1.0 / SAMPLES, scalar2=None, op0=AT.mult)
            v.tensor_scalar(out=sh, in0=bh, scalar1=1.0 / SAMPLES, scalar2=None, op0=AT.mult)
            v.tensor_scalar(out=rw01, in0=rw, scalar1=TRANS_STD, scalar2=None, op0=AT.mult)
            v.tensor_scalar(out=rh01, in0=rh, scalar1=TRANS_STD, scalar2=None, op0=AT.mult)

            # ---------------- stage A: per-sample coords ----------------
            w_t = cpool.tile([P, NSB], f32, tag="w")
            v.tensor_scalar(out=w_t[:], in0=pwg[:], scalar1=bw, scalar2=None, op0=AT.mult)
            v.scalar_tensor_tensor(out=w_t[:], in0=iwg[:], scalar=sw, in1=w_t[:], op0=AT.mult, op1=AT.add)
            v.scalar_tensor_tensor(out=w_t[:], in0=expand49(off[:, 0:NBIN]), scalar=rw01, in1=w_t[:], op0=AT.mult, op1=AT.add)
            v.tensor_scalar(out=w_t[:], in0=w_t[:], scalar1=rsw, scalar2=None, op0=AT.add)

            h_t = cpool.tile([P, NSB], f32, tag="h")
            v.tensor_scalar(out=h_t[:], in0=phg[:], scalar1=bh, scalar2=None, op0=AT.mult)
            v.scalar_tensor_tensor(out=h_t[:], in0=ihg[:], scalar=sh, in1=h_t[:], op0=AT.mult, op1=AT.add)
            v.scalar_tensor_tensor(out=h_t[:], in0=expand49(off[:, NBIN:2 * NBIN]), scalar=rh01, in1=h_t[:], op0=AT.mult, op1=AT.add)
            v.tensor_scalar(out=h_t[:], in0=h_t[:], scalar1=rsh, scalar2=None, op0=AT.add)

            vm = cpool.tile([P, NSB], f32, tag="vm")
            t1 = tp.tile([P, NSB], f32, tag="t1")
            v.tensor_scalar(out=vm[:], in0=w_t[:], scalar1=-0.5, scalar2=None, op0=AT.is_ge)
            v.tensor_scalar(out=t1[:], in0=w_t[:], scalar1=float(W) - 0.5, scalar2=None, op0=AT.is_le)
            v.tensor_tensor(out=vm[:], in0=vm[:], in1=t1[:], op=AT.mult)
            v.tensor_scalar(out=t1[:], in0=h_t[:], scalar1=-0.5, scalar2=None, op0=AT.is_ge)
            v.tensor_tensor(out=vm[:], in0=vm[:], in1=t1[:], op=AT.mult)
            v.tensor_scalar(out=t1[:], in0=h_t[:], scalar1=float(H) - 0.5, scalar2=None, op0=AT.is_le)
            v.tensor_tensor(out=vm[:], in0=vm[:], in1=t1[:], op=AT.mult)

            wc = cpool.tile([P, NSB], f32, tag="wc")
            v.tensor_scalar(out=wc[:], in0=w_t[:], scalar1=0.0, scalar2=float(W - 1), op0=AT.max, op1=AT.min)
            hc = cpool.tile([P, NSB], f32, tag="hc")
            v.tensor_scalar(out=hc[:], in0=h_t[:], scalar1=0.0, scalar2=float(H - 1), op0=AT.max, op1=AT.min)

            x1f = cpool.tile([P, NSB], f32, tag="x1f")
            v.tensor_scalar(out=x1f[:], in0=wc[:], scalar1=MAGIC, scalar2=-MAGIC, op0=AT.add, op1=AT.add)
            v.tensor_tensor(out=t1[:], in0=x1f[:], in1=wc[:], op=AT.is_gt)
            v.tensor_tensor(out=x1f[:], in0=x1f[:], in1=t1[:], op=AT.subtract)
            y1f = cpool.tile([P, NSB], f32, tag="y1f")
            v.tensor_scalar(out=y1f[:], in0=hc[:], scalar1=MAGIC, scalar2=-MAGIC, op0=AT.add, op1=AT.add)
            v.tensor_tensor(out=t1[:], in0=y1f[:], in1=hc[:], op=AT.is_gt)
            v.tensor_tensor(out=y1f[:], in0=y1f[:], in1=t1[:], op=AT.subtract)

            dx = cpool.tile([P, NSB], f32, tag="dx")
            v.tensor_tensor(out=dx[:], in0=wc[:], in1=x1f[:], op=AT.subtract)
            dy = cpool.tile([P, NSB], f32, tag="dy")
            v.tensor_tensor(out=dy[:], in0=hc[:], in1=y1f[:], op=AT.subtract)

            # per-sample corner weights [P, (bin, ih, iw)]
            w11 = cpool.tile([P, NSB], f32, tag="w11")
            w12 = cpool.tile([P, NSB], f32, tag="w12")
            w21 = cpool.tile([P, NSB], f32, tag="w21")
            w22 = cpool.tile([P, NSB], f32, tag="w22")
            omdx = tp.tile([P, NSB], f32, tag="omdx")
            vdy = tp.tile([P, NSB], f32, tag="vdy")
            vomdy = tp.tile([P, NSB], f32, tag="vomdy")
            v.tensor_scalar(out=omdx[:], in0=dx[:], scalar1=1.0, scalar2=-1.0, op0=AT.subtract, op1=AT.mult)
            v.tensor_tensor(out=vdy[:], in0=vm[:], in1=dy[:], op=AT.mult)
            v.tensor_tensor(out=vomdy[:], in0=vm[:], in1=vdy[:], op=AT.subtract)
            v.tensor_tensor(out=w11[:], in0=omdx[:], in1=vomdy[:], op=AT.mult)
            v.tensor_tensor(out=w12[:], in0=dx[:], in1=vomdy[:], op=AT.mult)
            v.tensor_tensor(out=w21[:], in0=omdx[:], in1=vdy[:], op=AT.mult)
            v.tensor_tensor(out=w22[:], in0=dx[:], in1=vdy[:], op=AT.mult)

            # count -> scale
            cnt = cpool.tile([P, NBIN], f32, tag="cnt")
            v.tensor_reduce(out=cnt[:], in_=vm[:].rearrange("p (b s) -> p b s", b=NBIN), axis=mybir.AxisListType.X, op=AT.add)
            scl = sa.tile([P, NBIN], f32, tag="scl")
            mx = tp.tile([P, NBIN], f32, tag="mx")
            v.tensor_scalar(out=mx[:], in0=cnt[:], scalar1=1.0, scalar2=None, op0=AT.max)
            v.reciprocal(out=mx[:], in_=mx[:])
            v.tensor_scalar(out=scl[:], in0=cnt[:], scalar1=0.0, scalar2=None, op0=AT.is_gt)
            v.tensor_tensor(out=scl[:], in0=scl[:], in1=mx[:], op=AT.mult)

            # ---------------- window base + coefficient planes ----------------
            def colsl(t, start, *dims):
                a = t[:]
                return bass.AP(a.tensor, a.offset + start, [list(a.ap[0])] + [list(d) for d in dims])

            # xbase [P, NBIN] = min(x1f[:, bin*16], W-WIN)
            xb = cpool.tile([P, NBIN], f32, tag="xb")
            v.tensor_scalar(out=xb[:], in0=colsl(x1f, 0, [NS, NBIN]),
                            scalar1=float(W - WIN), scalar2=None, op0=AT.min)
            # di [P, (bin, iw)] = x1f[:, bin*16 + iw] - xb  (ih=0 slice)
            di = cpool.tile([P, NBIN * SAMPLES], f32, tag="di")
            v.tensor_tensor(out=di[:].rearrange("p (b i) -> p b i", b=NBIN),
                            in0=colsl(x1f, 0, [NS, NBIN], [1, SAMPLES]),
                            in1=colsl(xb, 0, [1, NBIN], [0, SAMPLES]),
                            op=AT.subtract)

            # coefficient planes C_top/C_bot [P, (bin, ih, q)] col = bin*24 + ih*6 + q
            ctop = sa.tile([P, NBIN * SAMPLES * QW], f32, tag="ctop")
            cbot = sa.tile([P, NBIN * SAMPLES * QW], f32, tag="cbot")
            mq = tp.tile([P, NBIN * SAMPLES], f32, tag="mq")
            tmp = tp.tile([P, NSB], f32, tag="tmp")
            red1 = tp.tile([P, NBIN * SAMPLES], f32, tag="red1")
            red2 = tp.tile([P, NBIN * SAMPLES], f32, tag="red2")
            mqm1 = tp.tile([P, NBIN * SAMPLES], f32, tag="mqm1")

            def mq_exp(m):
                a = m[:]
                return bass.AP(a.tensor, a.offset, [list(a.ap[0]), [SAMPLES, NBIN], [0, SAMPLES], [1, SAMPLES]])

            w4 = lambda t: t[:].rearrange("p (b i j) -> p b i j", b=NBIN, i=SAMPLES)
            r3 = lambda t: t[:].rearrange("p (b i) -> p b i", b=NBIN)

            for q in range(QW):
                v.tensor_scalar(out=mq[:], in0=di[:], scalar1=float(q), scalar2=None, op0=AT.is_equal)
                if q > 0:
                    v.tensor_scalar(out=mqm1[:], in0=di[:], scalar1=float(q - 1), scalar2=None, op0=AT.is_equal)
                for cc_t, wA, wB in ((ctop, w11, w12), (cbot, w21, w22)):
                    v.tensor_tensor(out=w4(tmp), in0=mq_exp(mq), in1=w4(wA), op=AT.mult)
                    v.tensor_reduce(out=red1[:], in_=w4(tmp), axis=mybir.AxisListType.X, op=AT.add)
                    c_dst = colsl(cc_t, q, [SAMPLES * QW, NBIN], [QW, SAMPLES])
                    if q == 0:
                        v.tensor_copy(out=c_dst, in_=r3(red1))
                    else:
                        v.tensor_tensor(out=w4(tmp), in0=mq_exp(mqm1), in1=w4(wB), op=AT.mult)
                        v.tensor_reduce(out=red2[:], in_=w4(tmp), axis=mybir.AxisListType.X, op=AT.add)
                        v.tensor_tensor(out=c_dst, in0=r3(red1), in1=r3(red2), op=AT.add)

            # ---------------- gather row indices ----------------
            # itop [P, (bin, ih)] = base + y1f(bin, ih, iw=0)*W + xb(bin)
            itb = cpool.tile([P, NBIN * SAMPLES], f32, tag="itb")
            v.tensor_scalar(out=r3(itb),
                            in0=colsl(y1f, 0, [NS, NBIN], [SAMPLES, SAMPLES]),
                            scalar1=float(W), scalar2=basec[:, 0:1], op0=AT.mult, op1=AT.add)
            v.tensor_tensor(out=r3(itb), in0=r3(itb),
                            in1=colsl(xb, 0, [1, NBIN], [0, SAMPLES]), op=AT.add)
            ibb = cpool.tile([P, NBIN * SAMPLES], f32, tag="ibb")
            v.tensor_scalar(out=ibb[:], in0=itb[:], scalar1=float(W), scalar2=float(TBL_ROWS - 1), op0=AT.add, op1=AT.min)

            # IDXNAT [P, (bin, ih, t)] col = bin*8 + ih*2 + t
            idxnat = sa.tile([P, NBIN * NBLK], f32, tag="idxnat")
            v.tensor_copy(out=colsl(idxnat, 0, [NBLK, NBIN], [2, SAMPLES]), in_=r3(itb))
            v.tensor_copy(out=colsl(idxnat, 1, [NBLK, NBIN], [2, SAMPLES]), in_=r3(ibb))

            ephemeral.__exit__(None, None, None)

            feat_ap = bass.AP(feat, 0, [[EC, TBL_ROWS + TBL_PAD - WIN], [1, ES]])

            accA = accp.tile([P, C], f32, tag="accA")
            accB = accp.tile([P, C], f32, tag="accB")
            accC = accp.tile([P, C], f32, tag="accC")
            accD = accp.tile([P, C], f32, tag="accD")
            accs = [accA, accB, accC, accD]

            # ---------------- stage B: per bin ----------------
            NW = NBLK * 8  # wrapped cols = NIDX/16 = 64
            g_keep = None
            for bin_i in range(NBIN):
                rp_t = rp.tile([P, NW], f32, tag="rp")
                ix = idxnat[:, bin_i * NBLK:(bin_i + 1) * NBLK]
                in0 = bass.AP(ix.tensor, ix.offset, [list(ix.ap[0]), [1, NBLK], [0, 8]])
                qv = qm_psum[:]
                in1 = bass.AP(qv.tensor, qv.offset, [list(qv.ap[0]), [8, NBLK], [1, 8]])
                ov = rp_t[:]
                outap = bass.AP(ov.tensor, ov.offset, [list(ov.ap[0]), [8, NBLK], [1, 8]])
                v.tensor_tensor(out=outap, in0=in0, in1=in1, op=AT.mult)
                psum_t = pp.tile([P, NW], f32, tag="psx")
                nc.tensor.matmul(out=psum_t[:], lhsT=mod16[:], rhs=rp_t[:], start=True, stop=True)
                idx16 = ip.tile([P, NW], mybir.dt.int16, tag="idx16")
                v.tensor_copy(out=idx16[:], in_=psum_t[:])

                HBK = NBLK // 2  # 4 blocks per half-gather
                if mode != "nogather" or bin_i == 0:
                    g_halves = []
                    for hf in range(2):
                        g_t = gp.tile([P, HBK * ES], fdt, tag="g")
                        nc.gpsimd.dma_gather(
                            out_ap=g_t[:].rearrange("p (b e) -> p b e", b=HBK),
                            in_ap=feat_ap, idxs_ap=idx16[:, hf * 32:(hf + 1) * 32],
                            num_idxs=NIDX // 2, num_idxs_reg=NIDX // 2, elem_size=ES, elem_step=EC,
                        )
                        g_halves.append(g_t)
                    g_keep = g_halves
                else:
                    g_halves = g_keep
                gvs = [g[:].rearrange("p (b e) -> p b e", b=HBK) for g in g_halves]

                if mode != "nostt":
                    for ihi in range(SAMPLES):
                        gv = gvs[ihi // 2]
                        for t in range(2):
                            blk = (ihi % 2) * 2 + t
                            cc = ctop if t == 0 else cbot
                            chain = accs[(ihi % 2) * 2 + t]
                            for q in range(QW):
                                colw = bin_i * (SAMPLES * QW) + ihi * QW + q
                                first = (ihi < 2 and q == 0)
                                if first:
                                    v.tensor_scalar(out=chain[:], in0=gv[:, blk, q * EC:(q + 1) * EC],
                                                    scalar1=cc[:, colw:colw + 1], scalar2=None, op0=AT.mult)
                                else:
                                    v.scalar_tensor_tensor(out=chain[:], in0=gv[:, blk, q * EC:(q + 1) * EC],
                                                           scalar=cc[:, colw:colw + 1], in1=chain[:],
                                                           op0=AT.mult, op1=AT.add)
                elif bin_i == 0:
                    for a in accs:
                        v.memset(a[:], 0.0)

                stg = stp.tile([P, C], f32, tag="stg")
                v.tensor_tensor(out=accA[:], in0=accA[:], in1=accB[:], op=AT.add)
                v.tensor_tensor(out=accC[:], in0=accC[:], in1=accD[:], op=AT.add)
                v.tensor_tensor(out=accA[:], in0=accA[:], in1=accC[:], op=AT.add)
                v.tensor_scalar(out=stg[:], in0=accA[:], scalar1=scl[:, bin_i:bin_i + 1], scalar2=None, op0=AT.mult)
                nc.gpsimd.dma_start(out_d[:, bin_i * C:(bin_i + 1) * C], stg[:])

    nc.compile()
    return nc


_PROGRAM = None


def _prepare(input, rois, offset):
    input = np.asarray(input)
    rois = np.asarray(rois)
    offset = np.asarray(offset)

    feat_cl = np.ascontiguousarray(np.transpose(input, (0, 2, 3, 1))).reshape(B, ROWS_PER_IMG, C)
    ftype = np.float16 if FEAT_FP16 else np.float32
    featx = feat_cl.astype(ftype) if FEAT_FP16 else feat_cl
    batch_idx = rois[:, 0].astype(np.int64)
    core_ids, core_base = _assign_rois(batch_idx)

    bins = np.arange(NBIN)
    ss = np.arange(NS)
    pw = np.repeat(bins % POOLED, NS).astype(np.float32)
    ph = np.repeat(bins // POOLED, NS).astype(np.float32)
    iw = np.tile(ss % SAMPLES, NBIN).astype(np.float32)
    ih = np.tile(ss // SAMPLES, NBIN).astype(np.float32)
    pwg = np.broadcast_to(pw, (P, NBIN * NS)).copy()
    phg = np.broadcast_to(ph, (P, NBIN * NS)).copy()
    iwg = np.broadcast_to(iw, (P, NBIN * NS)).copy()
    ihg = np.broadcast_to(ih, (P, NBIN * NS)).copy()
    mod16 = (np.arange(P)[:, None] % 16 == np.arange(P)[None, :] % 16).astype(np.float32)
    qmask = (np.arange(P)[:, None] // 16 == np.arange(P)[None, :] % 8).astype(np.float32)

    in_maps = []
    for c in range(B):
        ids = core_ids[c]
        tbl = np.empty((TBL_ROWS + TBL_PAD, EC), ftype)
        tbl[:ROWS_PER_IMG] = featx[c]
        tbl[ROWS_PER_IMG:TBL_ROWS] = featx[(c + 1) % B]
        tbl[TBL_ROWS:] = 0.0
        in_maps.append({
            "feat": tbl,
            "rois": np.ascontiguousarray(rois[ids]),
            "off": np.ascontiguousarray(offset[ids].reshape(len(ids), -1)),
            "base": core_base[c][:, None].copy(),
            "pwg": pwg, "phg": phg, "iwg": iwg, "ihg": ihg,
            "mod16": mod16, "qmask": qmask,
        })
    return in_maps, core_ids


def _collect(res, core_ids):
    out = np.empty((K, C, POOLED, POOLED), np.float32)
    for c in range(B):
        o = res.results[c]["out"].reshape(P, NBIN, C)
        out[core_ids[c]] = np.transpose(o, (0, 2, 1)).reshape(P, C, POOLED, POOLED)
    return out


def kernel(input, rois, offset):
    global _PROGRAM
    from concourse.bass_utils import run_bass_kernel_spmd

    in_maps, core_ids = _prepare(input, rois, offset)
    if _PROGRAM is None:
        _PROGRAM = _build_program()
    res = run_bass_kernel_spmd(_PROGRAM, in_maps, core_ids=list(range(8)))
    return _collect(res, core_ids)


def run_traced(ins):
    global _PROGRAM
    from concourse.bass_utils import run_bass_kernel_spmd

    in_maps, core_ids = _prepare(ins["input"], ins["rois"], ins["offset"])
    if _PROGRAM is None:
        _PROGRAM = _build_program()
    res = run_bass_kernel_spmd(_PROGRAM, in_maps, core_ids=list(range(8)), trace=True)
    return res.exec_time_ns, _collect(res, core_ids)


# revision 18
# speedup vs baseline: 1.4166x; 1.4166x over previous
"""DCNv2 deformable PS-RoI pooling on 8 Trainium2 cores.

Sharding: each core holds a 2-image slice of the feature map (images c and
(c+1)%8, channels-last, stacked into a 32768-row pixel table -> int16
indexable) and exactly 128 RoIs assigned by a cyclic load balancer.

Device kernel (per core, RoIs on partitions):
- stage A (DVE): per-sample coords, validity, bilinear weights; per-bin the
  16 samples form a 4x4 tensor grid (x positions shared across sample rows),
  so all samples of one (bin, sample-row) live in one 6-pixel window; the
  x-interpolation folds into 6 coefficient planes C_top/C_bot built by
  iota-compare + reduce.
- per bin: a tiny PE matmul permutes the 8 window row-indices per RoI into
  dma_gather's wrapped-int16 layout; one dma_gather (1024 idx x 6px rows)
  pulls the data; 48 scalar_tensor_tensor ops accumulate coeff * window
  slices into 4 accumulator chains; scale by 1/count; DMA out.
"""

import sys

sys.path.insert(0, "/opt/trn_rl_repo")

import numpy as np

SPATIAL_SCALE = 0.0625
POOLED = 7
SAMPLES = 4
TRANS_STD = 0.1
B, C, H, W = 8, 256, 128, 128
K = 1024
NBIN = POOLED * POOLED          # 49
NS = SAMPLES * SAMPLES          # 16 samples per bin
P = 128                         # partitions == rois per core
NBLK = SAMPLES * 2              # 8 gather rows per bin (4 sample-rows x top/bot)
NIDX = P * NBLK                 # 1024 indices per gather (DGE per-op limit)
WIN = 6                         # window width in pixels
QW = WIN                        # coefficient slots
ROWS_PER_IMG = H * W            # 16384
TBL_ROWS = 2 * ROWS_PER_IMG     # 32768
TBL_PAD = 64
EC = C                          # elems per pixel row
ES = WIN * EC                   # gather elem_size: 6 pixels
MAGIC = 8388608.0               # 2^23
FEAT_FP16 = False


def _assign_rois(batch_idx):
    """Cyclic load balancer: core c serves images c,(c+1)%8; exactly P rois per core."""
    n = np.bincount(batch_idx, minlength=B).astype(np.int64)
    d = n - P
    prefix = np.concatenate([[0], np.cumsum(d)])[:-1]
    s0 = max(0, int(prefix.max()))
    s = s0 - prefix
    assert np.all(s >= 0) and np.all(s <= n), (n, s)
    ids_by_img = [np.where(batch_idx == b)[0] for b in range(B)]
    core_ids = []
    core_base = []
    for c in range(B):
        nxt = (c + 1) % B
        own = ids_by_img[c][s[c]:]
        spill = ids_by_img[nxt][: s[nxt]]
        ids = np.concatenate([own, spill])
        base = np.concatenate(
            [np.zeros(len(own), np.float32), np.full(len(spill), float(ROWS_PER_IMG), np.float32)]
        )
        assert len(ids) == P, (c, len(ids))
        core_ids.append(ids)
        core_base.append(base)
    return core_ids, core_base


def _build_program(mode="full"):
    import concourse.bass as bass
    import concourse.tile as tile
    from concourse import mybir, bacc

    f32 = mybir.dt.float32
    f16 = mybir.dt.float16
    fdt = f16 if FEAT_FP16 else f32
    AT = mybir.AluOpType

    nc = bacc.Bacc("TRN2", target_bir_lowering=False, debug=False)
    feat = nc.dram_tensor("feat", [TBL_ROWS + TBL_PAD, EC], fdt, kind="ExternalInput")
    rois_in = nc.dram_tensor("rois", [P, 5], f32, kind="ExternalInput")
    off_in = nc.dram_tensor("off", [P, 2 * NBIN], f32, kind="ExternalInput")
    base_in = nc.dram_tensor("base", [P, 1], f32, kind="ExternalInput")
    pwg_in = nc.dram_tensor("pwg", [P, NBIN * NS], f32, kind="ExternalInput")
    phg_in = nc.dram_tensor("phg", [P, NBIN * NS], f32, kind="ExternalInput")
    iwg_in = nc.dram_tensor("iwg", [P, NBIN * NS], f32, kind="ExternalInput")
    ihg_in = nc.dram_tensor("ihg", [P, NBIN * NS], f32, kind="ExternalInput")
    mod16_in = nc.dram_tensor("mod16", [P, P], f32, kind="ExternalInput")
    qmask_in = nc.dram_tensor("qmask", [P, P], f32, kind="ExternalInput")
    out_d = nc.dram_tensor("out", [P, NBIN * C], f32, kind="ExternalOutput")

    NSB = NBIN * NS  # 784

    def expand49(ap_2d):
        a = ap_2d
        return bass.AP(a.tensor, a.offset, list(a.ap[:1]) + [[a.ap[1][0], NBIN], [0, NS]])

    with tile.TileContext(nc) as tc:
        with (
            tc.tile_pool(name="keep", bufs=1) as sa,
            tc.tile_pool(name="gather", bufs=6 if FEAT_FP16 else 4) as gp,
            tc.tile_pool(name="idx", bufs=6) as ip,
            tc.tile_pool(name="rp", bufs=4) as rp,
            tc.tile_pool(name="acc", bufs=1) as accp,
            tc.tile_pool(name="stage", bufs=4) as stp,
            tc.tile_pool(name="psum", bufs=6, space="PSUM") as pp,
            tc.tile_pool(name="psumc", bufs=1, space="PSUM") as ppc,
        ):
            v = nc.vector

            mod16 = sa.tile([P, P], f32, tag="mod16")
            nc.gpsimd.dma_start(mod16[:], mod16_in[:, :])
            qm_psum = ppc.tile([P, P], f32, tag="qm")

            ephemeral = tc.tile_pool(name="eph", bufs=1)
            cpool = tp = ephemeral.__enter__()

            qm_sb = cpool.tile([P, P], f32, tag="qmsb")
            nc.gpsimd.dma_start(qm_sb[:], qmask_in[:, :])
            v.tensor_copy(out=qm_psum[:], in_=qm_sb[:])

            pwg = cpool.tile([P, NSB], f32, tag="pwg")
            nc.gpsimd.dma_start(pwg[:], pwg_in[:, :])
            phg = cpool.tile([P, NSB], f32, tag="phg")
            nc.gpsimd.dma_start(phg[:], phg_in[:, :])
            iwg = cpool.tile([P, NSB], f32, tag="iwg")
            nc.gpsimd.dma_start(iwg[:], iwg_in[:, :])
            ihg = cpool.tile([P, NSB], f32, tag="ihg")
            nc.gpsimd.dma_start(ihg[:], ihg_in[:, :])
            rois = cpool.tile([P, 5], f32, tag="rois")
            nc.gpsimd.dma_start(rois[:], rois_in[:, :])
            off = cpool.tile([P, 2 * NBIN], f32, tag="off")
            nc.gpsimd.dma_start(off[:], off_in[:, :])
            basec = cpool.tile([P, 1], f32, tag="basec")
            nc.gpsimd.dma_start(basec[:], base_in[:, :])

            # ---------------- stage A: per-roi scalars ----------------
            S = SPATIAL_SCALE
            sc1 = cpool.tile([P, 16], f32, tag="sc1")

            rsw = sc1[:, 0:1]; rsh = sc1[:, 1:2]; rew = sc1[:, 2:3]; reh = sc1[:, 3:4]
            rw = sc1[:, 4:5]; rh = sc1[:, 5:6]; bw = sc1[:, 6:7]; bh = sc1[:, 7:8]
            sw = sc1[:, 8:9]; sh = sc1[:, 9:10]; rw01 = sc1[:, 10:11]; rh01 = sc1[:, 11:12]

            def rnd(dst, src_col):
                v.tensor_scalar(out=dst, in0=src_col, scalar1=0.5 + MAGIC, scalar2=-MAGIC, op0=AT.add, op1=AT.add)
                v.tensor_scalar(out=dst, in0=dst, scalar1=S, scalar2=-0.5, op0=AT.mult, op1=AT.add)

            rnd(rsw, rois[:, 1:2])
            rnd(rsh, rois[:, 2:3])
            v.tensor_scalar(out=rew, in0=rois[:, 3:4], scalar1=0.5 + MAGIC, scalar2=-MAGIC, op0=AT.add, op1=AT.add)
            v.tensor_scalar(out=rew, in0=rew, scalar1=S, scalar2=S - 0.5, op0=AT.mult, op1=AT.add)
            v.tensor_scalar(out=reh, in0=rois[:, 4:5], scalar1=0.5 + MAGIC, scalar2=-MAGIC, op0=AT.add, op1=AT.add)
            v.tensor_scalar(out=reh, in0=reh, scalar1=S, scalar2=S - 0.5, op0=AT.mult, op1=AT.add)
            v.tensor_tensor(out=rw, in0=rew, in1=rsw, op=AT.subtract)
            v.tensor_scalar(out=rw, in0=rw, scalar1=0.1, scalar2=None, op0=AT.max)
            v.tensor_tensor(out=rh, in0=reh, in1=rsh, op=AT.subtract)
            v.tensor_scalar(out=rh, in0=rh, scalar1=0.1, scalar2=None, op0=AT.max)
            v.tensor_scalar(out=bw, in0=rw, scalar1=1.0 / POOLED, scalar2=None, op0=AT.mult)
            v.tensor_scalar(out=bh, in0=rh, scalar1=1.0 / POOLED, scalar2=None, op0=AT.mult)
            v.tensor_scalar(out=sw, in0=bw, scalar1=1.0 / SAMPLES, scalar2=None, op0=AT.mult)
            v.tensor_scalar(out=sh, in0=bh, scalar1=1.0 / SAMPLES, scalar2=None, op0=AT.mult)
            v.tensor_scalar(out=rw01, in0=rw, scalar1=TRANS_STD, scalar2=None, op0=AT.mult)
            v.tensor_scalar(out=rh01, in0=rh, scalar1=TRANS_STD, scalar2=None, op0=AT.mult)

            # ---------------- stage A: per-sample coords ----------------
            w_t = cpool.tile([P, NSB], f32, tag="w")
            v.tensor_scalar(out=w_t[:], in0=pwg[:], scalar1=bw, scalar2=None, op0=AT.mult)
            v.scalar_tensor_tensor(out=w_t[:], in0=iwg[:], scalar=sw, in1=w_t[:], op0=AT.mult, op1=AT.add)
            v.scalar_tensor_tensor(out=w_t[:], in0=expand49(off[:, 0:NBIN]), scalar=rw01, in1=w_t[:], op0=AT.mult, op1=AT.add)
            v.tensor_scalar(out=w_t[:], in0=w_t[:], scalar1=rsw, scalar2=None, op0=AT.add)

            h_t = cpool.tile([P, NSB], f32, tag="h")
            v.tensor_scalar(out=h_t[:], in0=phg[:], scalar1=bh, scalar2=None, op0=AT.mult)
            v.scalar_tensor_tensor(out=h_t[:], in0=ihg[:], scalar=sh, in1=h_t[:], op0=AT.mult, op1=AT.add)
            v.scalar_tensor_tensor(out=h_t[:], in0=expand49(off[:, NBIN:2 * NBIN]), scalar=rh01, in1=h_t[:], op0=AT.mult, op1=AT.add)
            v.tensor_scalar(out=h_t[:], in0=h_t[:], scalar1=rsh, scalar2=None, op0=AT.add)

            vm = cpool.tile([P, NSB], f32, tag="vm")
            t1 = tp.tile([P, NSB], f32, tag="t1")
            v.tensor_scalar(out=vm[:], in0=w_t[:], scalar1=-0.5, scalar2=None, op0=AT.is_ge)
            v.tensor_scalar(out=t1[:], in0=w_t[:], scalar1=float(W) - 0.5, scalar2=None, op0=AT.is_le)
            v.tensor_tensor(out=vm[:], in0=vm[:], in1=t1[:], op=AT.mult)
            v.tensor_scalar(out=t1[:], in0=h_t[:], scalar1=-0.5, scalar2=None, op0=AT.is_ge)
            v.tensor_tensor(out=vm[:], in0=vm[:], in1=t1[:], op=AT.mult)
            v.tensor_scalar(out=t1[:], in0=h_t[:], scalar1=float(H) - 0.5, scalar2=None, op0=AT.is_le)
            v.tensor_tensor(out=vm[:], in0=vm[:], in1=t1[:], op=AT.mult)

            wc = cpool.tile([P, NSB], f32, tag="wc")
            v.tensor_scalar(out=wc[:], in0=w_t[:], scalar1=0.0, scalar2=float(W - 1), op0=AT.max, op1=AT.min)
            hc = cpool.tile([P, NSB], f32, tag="hc")
            v.tensor_scalar(out=hc[:], in0=h_t[:], scalar1=0.0, scalar2=float(H - 1), op0=AT.max, op1=AT.min)

            x1f = cpool.tile([P, NSB], f32, tag="x1f")
            v.tensor_scalar(out=x1f[:], in0=wc[:], scalar1=MAGIC, scalar2=-MAGIC, op0=AT.add, op1=AT.add)
            v.tensor_tensor(out=t1[:], in0=x1f[:], in1=wc[:], op=AT.is_gt)
            v.tensor_tensor(out=x1f[:], in0=x1f[:], in1=t1[:], op=AT.subtract)
            y1f = cpool.tile([P, NSB], f32, tag="y1f")
            v.tensor_scalar(out=y1f[:], in0=hc[:], scalar1=MAGIC, scalar2=-MAGIC, op0=AT.add, op1=AT.add)
            v.tensor_tensor(out=t1[:], in0=y1f[:], in1=hc[:], op=AT.is_gt)
            v.tensor_tensor(out=y1f[:], in0=y1f[:], in1=t1[:], op=AT.subtract)

            dx = cpool.tile([P, NSB], f32, tag="dx")
            v.tensor_tensor(out=dx[:], in0=wc[:], in1=x1f[:], op=AT.subtract)
            dy = cpool.tile([P, NSB], f32, tag="dy")
            v.tensor_tensor(out=dy[:], in0=hc[:], in1=y1f[:], op=AT.subtract)

            # per-sample corner weights [P, (bin, ih, iw)]
            w11 = cpool.tile([P, NSB], f32, tag="w11")
            w12 = cpool.tile([P, NSB], f32, tag="w12")
            w21 = cpool.tile([P, NSB], f32, tag="w21")
            w22 = cpool.tile([P, NSB], f32, tag="w22")
            omdx = tp.tile([P, NSB], f32, tag="omdx")
            vdy = tp.tile([P, NSB], f32, tag="vdy")
            vomdy = tp.tile([P, NSB], f32, tag="vomdy")
            v.tensor_scalar(out=omdx[:], in0=dx[:], scalar1=1.0, scalar2=-1.0, op0=AT.subtract, op1=AT.mult)
            v.tensor_tensor(out=vdy[:], in0=vm[:], in1=dy[:], op=AT.mult)
            v.tensor_tensor(out=vomdy[:], in0=vm[:], in1=vdy[:], op=AT.subtract)
            v.tensor_tensor(out=w11[:], in0=omdx[:], in1=vomdy[:], op=AT.mult)
            v.tensor_tensor(out=w12[:], in0=dx[:], in1=vomdy[:], op=AT.mult)
            v.tensor_tensor(out=w21[:], in0=omdx[:], in1=vdy[:], op=AT.mult)
            v.tensor_tensor(out=w22[:], in0=dx[:], in1=vdy[:], op=AT.mult)

            # count -> scale
            cnt = cpool.tile([P, NBIN], f32, tag="cnt")
            v.tensor_reduce(out=cnt[:], in_=vm[:].rearrange("p (b s) -> p b s", b=NBIN), axis=mybir.AxisListType.X, op=AT.add)
            scl = sa.tile([P, NBIN], f32, tag="scl")
            mx = tp.tile([P, NBIN], f32, tag="mx")
            v.tensor_scalar(out=mx[:], in0=cnt[:], scalar1=1.0, scalar2=None, op0=AT.max)
            v.reciprocal(out=mx[:], in_=mx[:])
            v.tensor_scalar(out=scl[:], in0=cnt[:], scalar1=0.0, scalar2=None, op0=AT.is_gt)
            v.tensor_tensor(out=scl[:], in0=scl[:], in1=mx[:], op=AT.mult)

            # ---------------- window base + coefficient planes ----------------
            def colsl(t, start, *dims):
                a = t[:]
                return bass.AP(a.tensor, a.offset + start, [list(a.ap[0])] + [list(d) for d in dims])

            # xbase [P, NBIN] = min(x1f[:, bin*16], W-WIN)
            xb = cpool.tile([P, NBIN], f32, tag="xb")
            v.tensor_scalar(out=xb[:], in0=colsl(x1f, 0, [NS, NBIN]),
                            scalar1=float(W - WIN), scalar2=None, op0=AT.min)
            # di [P, (bin, iw)] = x1f[:, bin*16 + iw] - xb  (ih=0 slice)
            di = cpool.tile([P, NBIN * SAMPLES], f32, tag="di")
            v.tensor_tensor(out=di[:].rearrange("p (b i) -> p b i", b=NBIN),
                            in0=colsl(x1f, 0, [NS, NBIN], [1, SAMPLES]),
                            in1=colsl(xb, 0, [1, NBIN], [0, SAMPLES]),
                            op=AT.subtract)

            # coefficient planes C_top/C_bot [P, (bin, ih, q)] col = bin*24 + ih*6 + q
            ctop = sa.tile([P, NBIN * SAMPLES * QW], f32, tag="ctop")
            cbot = sa.tile([P, NBIN * SAMPLES * QW], f32, tag="cbot")
            mq = tp.tile([P, NBIN * SAMPLES], f32, tag="mq")
            tmp = tp.tile([P, NSB], f32, tag="tmp")
            red1 = tp.tile([P, NBIN * SAMPLES], f32, tag="red1")
            red2 = tp.tile([P, NBIN * SAMPLES], f32, tag="red2")
            mqm1 = tp.tile([P, NBIN * SAMPLES], f32, tag="mqm1")

            def mq_exp(m):
                a = m[:]
                return bass.AP(a.tensor, a.offset, [list(a.ap[0]), [SAMPLES, NBIN], [0, SAMPLES], [1, SAMPLES]])

            w4 = lambda t: t[:].rearrange("p (b i j) -> p b i j", b=NBIN, i=SAMPLES)
            r3 = lambda t: t[:].rearrange("p (b i) -> p b i", b=NBIN)

            for q in range(QW):
                v.tensor_scalar(out=mq[:], in0=di[:], scalar1=float(q), scalar2=None, op0=AT.is_equal)
                if q > 0:
                    v.tensor_scalar(out=mqm1[:], in0=di[:], scalar1=float(q - 1), scalar2=None, op0=AT.is_equal)
                for cc_t, wA, wB in ((ctop, w11, w12), (cbot, w21, w22)):
                    v.tensor_tensor(out=w4(tmp), in0=mq_exp(mq), in1=w4(wA), op=AT.mult)
                    v.tensor_reduce(out=red1[:], in_=w4(tmp), axis=mybir.AxisListType.X, op=AT.add)
                    c_dst = colsl(cc_t, q, [SAMPLES * QW, NBIN], [QW, SAMPLES])
                    if q == 0:
                        v.tensor_copy(out=c_dst, in_=r3(red1))
                    else:
                        v.tensor_tensor(out=w4(tmp), in0=mq_exp(mqm1), in1=w4(wB), op=AT.mult)
                        v.tensor_reduce(out=red2[:], in_=w4(tmp), axis=mybir.AxisListType.X, op=AT.add)
                        v.tensor_tensor(out=c_dst, in0=r3(red1), in1=r3(red2), op=AT.add)

            # ---------------- gather row indices ----------------
            # itop [P, (bin, ih)] = base + y1f(bin, ih, iw=0)*W + xb(bin)
            itb = cpool.tile([P, NBIN * SAMPLES], f32, tag="itb")
            v.tensor_scalar(out=r3(itb),
                            in0=colsl(y1f, 0, [NS, NBIN], [SAMPLES, SAMPLES]),
                            scalar1=float(W), scalar2=basec[:, 0:1], op0=AT.mult, op1=AT.add)
            v.tensor_tensor(out=r3(itb), in0=r3(itb),
                            in1=colsl(xb, 0, [1, NBIN], [0, SAMPLES]), op=AT.add)
            ibb = cpool.tile([P, NBIN * SAMPLES], f32, tag="ibb")
            v.tensor_scalar(out=ibb[:], in0=itb[:], scalar1=float(W), scalar2=float(TBL_ROWS - 1), op0=AT.add, op1=AT.min)

            # IDXNAT [P, (bin, ih, t)] col = bin*8 + ih*2 + t
            idxnat = sa.tile([P, NBIN * NBLK], f32, tag="idxnat")
            v.tensor_copy(out=colsl(idxnat, 0, [NBLK, NBIN], [2, SAMPLES]), in_=r3(itb))
            v.tensor_copy(out=colsl(idxnat, 1, [NBLK, NBIN], [2, SAMPLES]), in_=r3(ibb))

            ephemeral.__exit__(None, None, None)

            feat_ap = bass.AP(feat, 0, [[EC, TBL_ROWS + TBL_PAD - WIN], [1, ES]])

            accA = accp.tile([P, C], f32, tag="accA")
            accB = accp.tile([P, C], f32, tag="accB")
            accC = accp.tile([P, C], f32, tag="accC")
            accD = accp.tile([P, C], f32, tag="accD")
            accs = [accA, accB, accC, accD]

            # ---------------- stage B: per bin ----------------
            NW = NBLK * 8  # wrapped cols = NIDX/16 = 64
            g_keep = None
            for bin_i in range(NBIN):
                rp_t = rp.tile([P, NW], f32, tag="rp")
                ix = idxnat[:, bin_i * NBLK:(bin_i + 1) * NBLK]
                in0 = bass.AP(ix.tensor, ix.offset, [list(ix.ap[0]), [1, NBLK], [0, 8]])
                qv = qm_psum[:]
                in1 = bass.AP(qv.tensor, qv.offset, [list(qv.ap[0]), [8, NBLK], [1, 8]])
                ov = rp_t[:]
                outap = bass.AP(ov.tensor, ov.offset, [list(ov.ap[0]), [8, NBLK], [1, 8]])
                v.tensor_tensor(out=outap, in0=in0, in1=in1, op=AT.mult)
                psum_t = pp.tile([P, NW], f32, tag="psx")
                nc.tensor.matmul(out=psum_t[:], lhsT=mod16[:], rhs=rp_t[:], start=True, stop=True)
                idx16 = ip.tile([P, NW], mybir.dt.int16, tag="idx16")
                v.tensor_copy(out=idx16[:], in_=psum_t[:])

                HBK = NBLK // 2  # 4 blocks per half-gather
                if mode != "nogather" or bin_i == 0:
                    g_halves = []
                    for hf in range(2):
                        g_t = gp.tile([P, HBK * ES], fdt, tag="g")
                        nc.gpsimd.dma_gather(
                            out_ap=g_t[:].rearrange("p (b e) -> p b e", b=HBK),
                            in_ap=feat_ap, idxs_ap=idx16[:, hf * 32:(hf + 1) * 32],
                            num_idxs=NIDX // 2, num_idxs_reg=NIDX // 2, elem_size=ES, elem_step=EC,
                        )
                        g_halves.append(g_t)
                    g_keep = g_halves
                else:
                    g_halves = g_keep
                gvs = [g[:].rearrange("p (b e) -> p b e", b=HBK) for g in g_halves]

                if mode != "nostt":
                    for ihi in range(SAMPLES):
                        gv = gvs[ihi // 2]
                        for t in range(2):
                            blk = (ihi % 2) * 2 + t
                            cc = ctop if t == 0 else cbot
                            chain = accs[(ihi % 2) * 2 + t]
                            for q in range(QW):
                                colw = bin_i * (SAMPLES * QW) + ihi * QW + q
                                first = (ihi < 2 and q == 0)
                                if first:
                                    v.tensor_scalar(out=chain[:], in0=gv[:, blk, q * EC:(q + 1) * EC],
                                                    scalar1=cc[:, colw:colw + 1], scalar2=None, op0=AT.mult)
                                else:
                                    v.scalar_tensor_tensor(out=chain[:], in0=gv[:, blk, q * EC:(q + 1) * EC],
                                                           scalar=cc[:, colw:colw + 1], in1=chain[:],
                                                           op0=AT.mult, op1=AT.add)
                elif bin_i == 0:
                    for a in accs:
                        v.memset(a[:], 0.0)

                stg = stp.tile([P, C], f32, tag="stg")
                v.tensor_tensor(out=accA[:], in0=accA[:], in1=accB[:], op=AT.add)
                v.tensor_tensor(out=accC[:], in0=accC[:], in1=accD[:], op=AT.add)
                v.tensor_tensor(out=accA[:], in0=accA[:], in1=accC[:], op=AT.add)
                v.tensor_scalar(out=stg[:], in0=accA[:], scalar1=scl[:, bin_i:bin_i + 1], scalar2=None, op0=AT.mult)
                nc.gpsimd.dma_start(out_d[:, bin_i * C:(bin_i + 1) * C], stg[:])

    nc.compile()
    return nc


_PROGRAM = None


def _prepare(input, rois, offset):
    input = np.asarray(input)
    rois = np.asarray(rois)
    offset = np.asarray(offset)

    feat_cl = np.ascontiguousarray(np.transpose(input, (0, 2, 3, 1))).reshape(B, ROWS_PER_IMG, C)
    ftype = np.float16 if FEAT_FP16 else np.float32
    featx = feat_cl.astype(ftype) if FEAT_FP16 else feat_cl
    batch_idx = rois[:, 0].astype(np.int64)
    core_ids, core_base = _assign_rois(batch_idx)

    bins = np.arange(NBIN)
    ss = np.arange(NS)
    pw = np.repeat(bins % POOLED, NS).astype(np.float32)
    ph = np.repeat(bins // POOLED, NS).astype(np.float32)
    iw = np.tile(ss % SAMPLES, NBIN).astype(np.float32)
    ih = np.tile(ss // SAMPLES, NBIN).astype(np.float32)
    pwg = np.broadcast_to(pw, (P, NBIN * NS)).copy()
    phg = np.broadcast_to(ph, (P, NBIN * NS)).copy()
    iwg = np.broadcast_to(iw, (P, NBIN * NS)).copy()
    ihg = np.broadcast_to(ih, (P, NBIN * NS)).copy()
    mod16 = (np.arange(P)[:, None] % 16 == np.arange(P)[None, :] % 16).astype(np.float32)
    qmask = (np.arange(P)[:, None] // 16 == np.arange(P)[None, :] % 8).astype(np.float32)

    in_maps = []
    for c in range(B):
        ids = core_ids[c]
        tbl = np.empty((TBL_ROWS + TBL_PAD, EC), ftype)
        tbl[:ROWS_PER_IMG] = featx[c]
        tbl[ROWS_PER_IMG:TBL_ROWS] = featx[(c + 1) % B]
        tbl[TBL_ROWS:] = 0.0
        in_maps.append({
            "feat": tbl,
            "rois": np.ascontiguousarray(rois[ids]),
            "off": np.ascontiguousarray(offset[ids].reshape(len(ids), -1)),
            "base": core_base[c][:, None].copy(),
            "pwg": pwg, "phg": phg, "iwg": iwg, "ihg": ihg,
            "mod16": mod16, "qmask": qmask,
        })
    return in_maps, core_ids


def _collect(res, core_ids):
    out = np.empty((K, C, POOLED, POOLED), np.float32)
    for c in range(B):
        o = res.results[c]["out"].reshape(P, NBIN, C)
        out[core_ids[c]] = np.transpose(o, (0, 2, 1)).reshape(P, C, POOLED, POOLED)
    return out


def kernel(input, rois, offset):
    global _PROGRAM
    from concourse.bass_utils import run_bass_kernel_spmd

    in_maps, core_ids = _prepare(input, rois, offset)
    if _PROGRAM is None:
        _PROGRAM = _build_program()
    res = run_bass_kernel_spmd(_PROGRAM, in_maps, core_ids=list(range(8)))
    return _collect(res, core_ids)


def run_traced(ins):
    global _PROGRAM
    from concourse.bass_utils import run_bass_kernel_spmd

    in_maps, core_ids = _prepare(ins["input"], ins["rois"], ins["offset"])
    if _PROGRAM is None:
        _PROGRAM = _build_program()
    res = run_bass_kernel_spmd(_PROGRAM, in_maps, core_ids=list(range(8)), trace=True)
    return res.exec_time_ns, _collect(res, core_ids)
